# revision 1
# baseline (speedup 1.0000x reference)
"""Trainium2 Bass kernel for nn_GAT_Decoder (one decode step of a GAT decoder).

Strategy (per spec sharding hint): pure data parallel, batch sharded 8 ways
(32 batch elements per core), weights replicated.

The three O(B*N*D^2) projections (K, V, Kp) of the reference are eliminated
algebraically; only O(B*N*D) work streams through the PE:
  compat[b,h,n] = qhat[b,h,:] . E[b,n,:]   with qhat = state @ Wq_h @ Wk_h^T
  ctx[b,h,:]    = attn[b,h,:] @ E[b]                 (contract n first)
  o[b,C_h]      = ctx[b,h,:] @ Wv[:,C_h]
  phat          = o @ G^T,  G = Wk_ptr @ Wo^T        (weights folded on device)
  compat2[b,n]  = phat[b,:] . E[b,n,:]
Masked rows (~50%) are compacted out host-side (gather); masked/pad scores are
exactly 0, matching the reference's softmax(-inf) = 0.

Device layout: per subgroup of 4 batches, compat/compat2/ctx outputs are packed
at partition offsets {0,32,64,96} via tile_position, so softmax/tanh run as
full-128-partition vector ops. E^T is materialized per batch by PE transposes
(or DMA'd from a host-transposed copy when HOST_ET).
"""

import numpy as np

B, N, D, H = 256, 1000, 512, 8
HD = D // H
NCORES = 8
BPC = B // NCORES          # batches per core
SG = 4                     # batches per subgroup (partition packing factor)
ND = D // 128              # 4 contraction chunks

NORM_MHA = float(1.0 / np.sqrt(HD))
NORM_PTR = float(1.0 / np.sqrt(D))
MASKVAL = 1e30

# knobs (overridable for experiments)
HOST_ET = False            # True: DMA host-transposed E^T instead of PE transposes
USE_F32R = True            # float32r for the big E streams
TR_F32R = False            # float32r transposes (exactness verified by probe)


def _build(n_c, bpc, host_et, use_f32r, tr_f32r):
    from concourse import bacc
    import concourse.mybir as mybir
    import concourse.tile as tile
    from concourse.masks import make_identity

    dt = mybir.dt
    AF = mybir.ActivationFunctionType
    ALU = mybir.AluOpType
    nt = n_c // 128
    nh = n_c // 2              # free size of each matmul half (<=512)
    nsg = bpc // SG
    f32 = dt.float32
    if use_f32r == "bf16":
        sdt = dt.bfloat16
    elif use_f32r:
        sdt = dt.float32r
    else:
        sdt = f32

    pdt = dt.float32r if use_f32r else f32   # phase-0 dtype
    nc = bacc.Bacc("TRN2", target_bir_lowering=False, debug=False)

    # ---- DRAM I/O ----
    Ec_d = nc.dram_tensor("Ec", [bpc, n_c, D], sdt, kind="ExternalInput")
    n_et = bpc if host_et is True else (bpc // 2 if host_et == "hybrid" else 0)
    ET_d = (nc.dram_tensor("EcT", [n_et, D, n_c], sdt, kind="ExternalInput")
            if n_et else None)
    mf8_d = nc.dram_tensor("maskf8", [bpc, 8, n_c], f32, kind="ExternalInput")
    mfp_d = nc.dram_tensor("maskfp", [nsg, 128, n_c], f32, kind="ExternalInput")
    wfc_d = nc.dram_tensor("W_fc", [D + 1, D], pdt, kind="ExternalInput")
    wfc1_d = nc.dram_tensor("W_fc1", [D, D], pdt, kind="ExternalInput")
    wq_d = nc.dram_tensor("Wq", [D, D], pdt, kind="ExternalInput")
    wkT_d = nc.dram_tensor("WkT", [D, D], pdt, kind="ExternalInput")
    wv_d = nc.dram_tensor("Wv", [D, D], sdt, kind="ExternalInput")
    woT_d = nc.dram_tensor("WoT", [D, D], pdt, kind="ExternalInput")
    wpT_d = nc.dram_tensor("WpT", [D, D], pdt, kind="ExternalInput")
    wlastT_d = nc.dram_tensor("wlastT", [128, ND], f32, kind="ExternalInput")
    e0T_d = nc.dram_tensor("E0T", [D, bpc], pdt, kind="ExternalInput")
    poolT_d = nc.dram_tensor("poolT", [D, bpc], pdt, kind="ExternalInput")
    dcrep_d = nc.dram_tensor("dcrep", [128, bpc], f32, kind="ExternalInput")
    out_d = nc.dram_tensor("scores", [bpc, n_c], f32, kind="ExternalOutput")

    def w_ap(d):  # [512,512] dram -> [128, 4, 512]
        return d[0:D, :].rearrange("(c p) d -> p c d", p=128)

    def tpos(row, col):
        return None if (row == 0 and col == 0) else (row, col)

    with tile.TileContext(nc) as tc:
        with tc.tile_pool(name="const", bufs=1) as constp, \
             tc.tile_pool(name="wmain", bufs=1) as wmain, \
             tc.tile_pool(name="small", bufs=1) as smallp:
            ident = constp.tile([128, 128], f32, tag="ident")
            make_identity(nc, ident[:])
            if use_f32r:
                identr = constp.tile([128, 128], sdt, tag="identr")
                nc.vector.tensor_copy(identr[:], ident[:])
            else:
                identr = ident

            # persistent
            wv_t = wmain.tile([128, ND, D], sdt, tag="wv")
            gt_t = wmain.tile([128, ND, D], sdt, tag="gt")
            qhatT = wmain.tile([128, ND, bpc, H], sdt, tag="qhatT")
            nc.gpsimd.dma_start(wv_t[:], w_ap(wv_d))

            # ---------- phase 0 ----------
            with tc.tile_pool(name="w0", bufs=1) as w0, \
                 tc.tile_pool(name="ps0", bufs=2, space="PSUM") as ps0:
                wfc_t = w0.tile([128, ND, D], pdt, tag="wfc")
                wfc1_t = w0.tile([128, ND, D], pdt, tag="wfc1")
                wq_t = w0.tile([128, ND, D], pdt, tag="wq")
                wkT_t = w0.tile([128, ND, D], pdt, tag="wkT")
                woT_t = w0.tile([128, ND, D], pdt, tag="woT")
                wpT_t = w0.tile([128, ND, D], pdt, tag="wpT")
                wlast_t = w0.tile([128, ND], f32, tag="wlast")
                e0T_t = w0.tile([128, ND, bpc], pdt, tag="e0T")
                poolT_t = w0.tile([128, ND, bpc], pdt, tag="poolT")
                dcrep_t = w0.tile([128, bpc], f32, tag="dcrep")
                stateT = w0.tile([128, ND, bpc], pdt, tag="stateT")
                qT_t = w0.tile([128, ND, bpc], pdt, tag="qT")
                nc.gpsimd.dma_start(wfc_t[:], w_ap(wfc_d))
                nc.gpsimd.dma_start(wfc1_t[:], w_ap(wfc1_d))
                nc.gpsimd.dma_start(wq_t[:], w_ap(wq_d))
                nc.gpsimd.dma_start(wkT_t[:], w_ap(wkT_d))
                nc.gpsimd.dma_start(woT_t[:], w_ap(woT_d))
                nc.gpsimd.dma_start(wpT_t[:], w_ap(wpT_d))
                nc.gpsimd.dma_start(wlast_t[:], wlastT_d[:])
                nc.gpsimd.dma_start(e0T_t[:], e0T_d[:].rearrange("(c p) b -> p c b", p=128))
                nc.gpsimd.dma_start(poolT_t[:], poolT_d[:].rearrange("(c p) b -> p c b", p=128))
                nc.gpsimd.dma_start(dcrep_t[:], dcrep_d[:])

                # G^T = WoT.T @ WpT, scaled by norm_ptr
                for c in range(ND):
                    g_ps = ps0.tile([128, D], f32, tag="g_ps")
                    for kc in range(ND):
                        nc.tensor.matmul(g_ps[:], woT_t[:, kc, 128 * c:128 * (c + 1)],
                                         wpT_t[:, kc, :], start=(kc == 0), stop=(kc == ND - 1))
                    nc.scalar.mul(gt_t[:, c, :], g_ps[:], NORM_PTR)

                # stateT
                for c in range(ND):
                    st_ps = ps0.tile([128, bpc], f32, tag="st_ps")
                    for kc in range(ND):
                        nc.tensor.matmul(st_ps[:], wfc_t[:, kc, 128 * c:128 * (c + 1)],
                                         e0T_t[:, kc, :], start=(kc == 0), stop=False)
                    for kc in range(ND):
                        nc.tensor.matmul(st_ps[:], wfc1_t[:, kc, 128 * c:128 * (c + 1)],
                                         poolT_t[:, kc, :], start=False, stop=(kc == ND - 1))
                    nc.vector.scalar_tensor_tensor(
                        stateT[:, c, :], dcrep_t[:], wlast_t[:, c:c + 1], st_ps[:],
                        op0=ALU.mult, op1=ALU.add)

                # QT (scaled by norm_mha)
                for c in range(ND):
                    q_ps = ps0.tile([128, bpc], f32, tag="q_ps")
                    for kc in range(ND):
                        nc.tensor.matmul(q_ps[:], wq_t[:, kc, 128 * c:128 * (c + 1)],
                                         stateT[:, kc, :], start=(kc == 0), stop=(kc == ND - 1))
                    nc.scalar.mul(qT_t[:, c, :], q_ps[:], NORM_MHA)

                # qhatT
                for h in range(H):
                    pb = 64 * (h % 2)
                    for c in range(ND):
                        qq = ps0.tile([128, bpc], f32, tag="qq")
                        nc.tensor.matmul(
                            qq[:], wkT_t[pb:pb + 64, h // 2, 128 * c:128 * (c + 1)],
                            qT_t[pb:pb + 64, h // 2, :], start=True, stop=True)
                        nc.vector.tensor_copy(qhatT[:, c, :, h], qq[:])

            # ---------- main loop ----------
            # Software-pipelined: stage A(b) = load/transpose/compat/softmax,
            # stage B(b) = expT/ctx/o/oT (needs A(b)'s softmax). B(b) is
            # emitted after A(b+1) so PE never waits on the softmax chain.
            # Per subgroup s, phatT + compat2 + pointer stage C(s) follows
            # B(last batch of s). All f32r matmuls write PSUM at partition 0;
            # strips are [8,n]/[1,n] (free-dim bound, lane count irrelevant).
            with tc.tile_pool(name="epool", bufs=4) as epool, \
                 tc.tile_pool(name="etpool", bufs=8) as etpool, \
                 tc.tile_pool(name="mfpool", bufs=3) as mfpool, \
                 tc.tile_pool(name="mfppool", bufs=2) as mfppool, \
                 tc.tile_pool(name="smpool", bufs=3) as smpool, \
                 tc.tile_pool(name="stg", bufs=2) as stg, \
                 tc.tile_pool(name="xtpool", bufs=1) as xtpool, \
                 tc.tile_pool(name="tp_ps", bufs=3, space="PSUM") as tp_ps, \
                 tc.tile_pool(name="big_ps", bufs=3, space="PSUM") as big_ps, \
                 tc.tile_pool(name="cx_ps", bufs=2, space="PSUM") as cx_ps:
                st = {}
                sgst = {}

                def emit_A1(bc):
                    sg = bc // SG
                    if bc % SG == 0:
                        mfp_t = mfppool.tile([128, n_c], f32, tag="mfp")
                        nc.sync.dma_start(mfp_t[:], mfp_d[sg])
                        oT_sb = smpool.tile([128, ND, SG], sdt, tag="oT_sb")
                        sgst[sg] = dict(mfp=mfp_t, oT=oT_sb, et={})
                    e_t = epool.tile([128, nt, D], sdt, tag="E")
                    nc.sync.dma_start(
                        e_t[:], Ec_d[bc].rearrange("(t p) d -> p t d", p=128))
                    et_t = etpool.tile([128, ND, n_c], sdt, tag="ET")
                    use_dma_et = (host_et is True) or (host_et == "hybrid" and bc % 2 == 1)
                    if use_dma_et:
                        ei = bc if host_et is True else bc // 2
                        nc.sync.dma_start(
                            et_t[:], ET_d[ei].rearrange("(c p) n -> p c n", p=128))
                    else:
                        for t in range(nt):
                            tp = tp_ps.tile([128, D], sdt, tag="tp")
                            for c in range(ND):
                                nc.tensor.transpose(
                                    tp[:, 128 * c:128 * (c + 1)],
                                    e_t[:, t, 128 * c:128 * (c + 1)],
                                    identr[:])
                            eng = nc.vector.tensor_copy if t % 2 == 0 else nc.scalar.copy
                            eng(et_t[:, :, 128 * t:128 * (t + 1)],
                                tp[:, :].rearrange("p (c x) -> p c x", c=ND))
                    sgst[sg]['et'][bc] = et_t
                    mf8_t = mfpool.tile([8, n_c], f32, tag="mf8")
                    nc.sync.dma_start(mf8_t[:], mf8_d[bc])
                    st[bc] = dict(e=e_t, et=et_t, mf8=mf8_t)

                def emit_A2(bc):
                    et_t, mf8_t = st[bc]['et'], st[bc]['mf8']
                    cp0 = big_ps.tile([8, nh], f32, tag="big")
                    cp1 = big_ps.tile([8, nh], f32, tag="big")
                    for half, cph in ((0, cp0), (1, cp1)):
                        for c in range(ND):
                            nc.tensor.matmul(
                                cph[:, :], qhatT[:, c, bc, :],
                                et_t[:, c, half * nh:(half + 1) * nh],
                                start=(c == 0), stop=(c == ND - 1))
                    u8 = stg.tile([8, n_c], f32, tag="u8")
                    nc.vector.tensor_sub(u8[:, 0:nh], cp0[:, :], mf8_t[:, 0:nh])
                    nc.vector.tensor_sub(u8[:, nh:n_c], cp1[:, :], mf8_t[:, nh:n_c])
                    s8 = smallp.tile([8, 1], f32, tag="s8")
                    ex8 = stg.tile([8, n_c], sdt, tag="ex8")
                    nc.scalar.activation(ex8[:, :], u8[:, :], AF.Exp,
                                         bias=0.0, scale=1.0, accum_out=s8[:])
                    r8 = smallp.tile([8, 1], f32, tag="r8")
                    nc.vector.reciprocal(r8[:], s8[:])
                    st[bc].update(ex8=ex8, r8=r8)

                def emit_expT(bc):
                    ex8 = st[bc]['ex8']
                    tpT = tp_ps.tile([128, D], sdt, tag="tp")
                    for t in range(nt):
                        nc.tensor.transpose(
                            tpT[:, 8 * t:8 * (t + 1)],
                            ex8[:, 128 * t:128 * (t + 1)], identr[0:8, 0:8])
                    expT_t = smpool.tile([128, nt, 8], sdt, tag="expT")
                    nc.vector.tensor_copy(
                        expT_t[:, :, :],
                        tpT[:, 0:8 * nt].rearrange("p (t x) -> p t x", t=nt))
                    st[bc]['expT'] = expT_t

                def emit_ctxu(bc):
                    # ctx = (unnormalized expT).T @ E, normalization folded into
                    # the PSUM->SBUF copy via scale=1/sum
                    expT_t, e_t, r8 = st[bc]['expT'], st[bc]['e'], st[bc]['r8']
                    cxp = cx_ps.tile([8, D], f32, tag="cx")
                    for t in range(nt):
                        nc.tensor.matmul(
                            cxp[:, :], expT_t[:, t, :], e_t[:, t, :],
                            start=(t == 0), stop=(t == nt - 1))
                    ctx8 = stg.tile([8, D], sdt, tag="ctx8")
                    nc.scalar.activation(ctx8[:, :], cxp[:, :], AF.Copy,
                                         bias=0.0, scale=r8[:])
                    st[bc]['ctx8'] = ctx8

                def emit_ctxT(bc):
                    ctx8 = st[bc]['ctx8']
                    tpC = tp_ps.tile([128, D], sdt, tag="tp")
                    for c in range(ND):
                        nc.tensor.transpose(
                            tpC[:, 8 * c:8 * (c + 1)],
                            ctx8[:, 128 * c:128 * (c + 1)], identr[0:8, 0:8])
                    ctxT_t = smpool.tile([128, ND, 8], sdt, tag="ctxT")
                    nc.vector.tensor_copy(
                        ctxT_t[:, :, :],
                        tpC[:, 0:8 * ND].rearrange("p (c x) -> p c x", c=ND))
                    st[bc]['ctxT'] = ctxT_t

                def emit_o(bc):
                    ctxT_t = st[bc]['ctxT']
                    op = cx_ps.tile([8, D], f32, tag="cx")
                    for c in range(ND):
                        nc.tensor.matmul(
                            op[:, :], ctxT_t[:, c, :], wv_t[:, c, :],
                            start=(c == 0), stop=(c == ND - 1))
                    o8 = stg.tile([8, D], sdt, tag="o8")
                    nc.scalar.copy(o8[:, :], op[:, :])
                    st[bc]['o8'] = o8

                def emit_oT(bc):
                    sg, j = bc // SG, bc % SG
                    o8 = st[bc]['o8']
                    tpO = tp_ps.tile([128, D], sdt, tag="tp")
                    for c in range(ND):
                        nc.tensor.transpose(
                            tpO[:, 8 * c:8 * (c + 1)],
                            o8[:, 128 * c:128 * (c + 1)], identr[0:8, 0:8])
                    oT_f = sgst[sg]['oT'][:, :, :].rearrange("p c j -> p (c j)")
                    for c in range(ND):
                        fi = c * SG + j
                        nc.vector.tensor_copy(
                            oT_f[0:64, fi:fi + 1], tpO[0:64, 10 * c:10 * c + 1])
                        nc.vector.tensor_copy(
                            oT_f[64:128, fi:fi + 1], tpO[64:128, 10 * c + 1:10 * c + 2])
                    del st[bc]

                def emit_C1(sg):
                    oT_sb = sgst[sg]['oT']
                    pp_ps = tp_ps.tile([128, 4 * SG], f32, tag="tp")
                    for c2 in range(ND):
                        for c in range(ND):
                            nc.tensor.matmul(
                                pp_ps[:, SG * c2:SG * (c2 + 1)],
                                gt_t[:, c, 128 * c2:128 * (c2 + 1)], oT_sb[:, c, :],
                                start=(c == 0), stop=(c == ND - 1))
                    phatT_t = smpool.tile([128, ND, SG], sdt, tag="phatT")
                    nc.vector.tensor_copy(
                        phatT_t[:, :, :],
                        pp_ps[:, :].rearrange("p (c j) -> p c j", c=ND))
                    sgst[sg]['phatT'] = phatT_t

                def emit_C2(sg):
                    phatT_t, mfp_t = sgst[sg]['phatT'], sgst[sg]['mfp']
                    ptr_pk = xtpool.tile([128, n_c], f32, tag="ptr")
                    for j in range(SG):
                        et_t = sgst[sg]['et'][SG * sg + j]
                        q0 = big_ps.tile([1, nh], f32, tag="big")
                        q1 = big_ps.tile([1, nh], f32, tag="big")
                        for half, qh in ((0, q0), (1, q1)):
                            for c in range(ND):
                                nc.tensor.matmul(
                                    qh[:, :], phatT_t[:, c, j:j + 1],
                                    et_t[:, c, half * nh:(half + 1) * nh],
                                    start=(c == 0), stop=(c == ND - 1))
                        p28 = stg.tile([1, n_c], f32, tag="p28")
                        nc.scalar.copy(p28[:, 0:nh], q0[:, :])
                        nc.scalar.copy(p28[:, nh:n_c], q1[:, :])
                        nc.sync.dma_start(ptr_pk[32 * j:32 * j + 1, :], p28[:, :])

                    tn_t = xtpool.tile([128, n_c], f32, tag="x1")
                    nc.scalar.activation(tn_t[:], ptr_pk[:, :], AF.Tanh)
                    x_t = xtpool.tile([128, n_c], f32, tag="x2")
                    nc.vector.scalar_tensor_tensor(
                        x_t[:, :], tn_t[:, :], 10.0, mfp_t[:, :],
                        op0=ALU.mult, op1=ALU.subtract)
                    e2_t = xtpool.tile([128, n_c], f32, tag="x1")
                    s2 = smallp.tile([128, 1], f32, tag="s2")
                    nc.scalar.activation(e2_t[:], x_t[:, :], AF.Exp,
                                         bias=0.0, scale=1.0, accum_out=s2[:])
                    r2 = smallp.tile([128, 1], f32, tag="r2")
                    nc.vector.reciprocal(r2[:], s2[:])
                    sc_t = xtpool.tile([128, n_c], f32, tag="x2")
                    nc.vector.tensor_scalar_mul(sc_t[:], e2_t[:], r2[:])
                    for j in range(SG):
                        nc.sync.dma_start(out_d[SG * sg + j], sc_t[32 * j:32 * j + 1, :])
                    del sgst[sg]

                # staggered rounds: lags hide every cross-engine latency
                for r in range(bpc + 5):
                    if r < bpc:
                        emit_A1(r)
                    if 0 <= r - 4 < bpc:
                        emit_oT(r - 4)
                        if (r - 4) % SG == SG - 1:
                            emit_C1((r - 4) // SG)
                    if 0 <= r - 1 < bpc:
                        emit_expT(r - 1)
                    if r < bpc:
                        emit_A2(r)
                    if 0 <= r - 4 < bpc and (r - 4) % SG == SG - 1:
                        emit_C2((r - 4) // SG)
                    if 0 <= r - 1 < bpc:
                        emit_ctxu(r - 1)
                    if 0 <= r - 2 < bpc:
                        emit_ctxT(r - 2)
                    if 0 <= r - 3 < bpc:
                        emit_o(r - 3)

    nc.finalize()
    return nc


def _host_prep(inputs, n_c=None):
    E = np.ascontiguousarray(inputs['encoder_inputs'], dtype=np.float32)
    mask = np.asarray(inputs['mask'])
    unm = (mask == 0)
    counts = unm.sum(axis=1)
    if n_c is None:
        n_c = max(512, int(np.ceil(counts.max() / 128) * 128))
    idx = np.zeros((B, n_c), dtype=np.int64)
    maskf = np.full((B, n_c), MASKVAL, dtype=np.float32)
    for b in range(B):
        ii = np.nonzero(unm[b])[0]
        k = min(len(ii), n_c)
        idx[b, :k] = ii[:k]
        maskf[b, :k] = 0.0
    Ec = np.take_along_axis(E, idx[:, :, None], axis=1)   # [B, n_c, D]
    return Ec, idx, counts, maskf, n_c


def _in_maps(inputs, Ec, maskf, n_c, bpc=BPC, host_et=False, bf16=False):
    nsg = bpc // SG
    Ec32 = Ec
    if bf16:
        import ml_dtypes
        Ec = Ec.astype(ml_dtypes.bfloat16)
    W_fc = np.asarray(inputs['W_fc'], dtype=np.float32)
    wlastT = np.ascontiguousarray(W_fc[D].reshape(ND, 128).T)        # [128, 4]
    wkT = np.ascontiguousarray(np.asarray(inputs['Wk_mha']).T)
    woT = np.ascontiguousarray(np.asarray(inputs['Wo']).T)
    wpT = np.ascontiguousarray(np.asarray(inputs['Wk_ptr']).T)
    pool = np.asarray(inputs['pool'], dtype=np.float32)
    dc = np.asarray(inputs['dynamic_capacity'], dtype=np.float32)
    # maskf expanded: [nsg, 128, n_c] per core (each batch row replicated x32)
    maps = []
    for i in range(NCORES):
        b0 = i * bpc
        mfe = np.repeat(maskf[b0:b0 + bpc], 32, axis=0).reshape(nsg, SG * 32, n_c)
        mf8 = np.repeat(maskf[b0:b0 + bpc], 8, axis=0).reshape(bpc, 8, n_c)
        m = {
            "Ec": np.ascontiguousarray(Ec[b0:b0 + bpc]),
            "maskf8": np.ascontiguousarray(mf8),
            "maskfp": np.ascontiguousarray(mfe),
            "W_fc": W_fc,
            "W_fc1": np.asarray(inputs['W_fc1'], dtype=np.float32),
            "Wq": np.asarray(inputs['Wq'], dtype=np.float32),
            "WkT": wkT,
            "Wv": (np.asarray(inputs['Wv'], dtype=np.float32).astype(__import__('ml_dtypes').bfloat16)
                   if bf16 else np.asarray(inputs['Wv'], dtype=np.float32)),
            "WoT": woT,
            "WpT": wpT,
            "wlastT": wlastT,
            "E0T": np.ascontiguousarray(Ec32[b0:b0 + bpc, 0, :].T),
            "poolT": np.ascontiguousarray(pool[b0:b0 + bpc].T),
            "dcrep": np.ascontiguousarray(np.broadcast_to(dc[b0:b0 + bpc, 0], (128, bpc))),
        }
        if host_et is True:
            m["EcT"] = np.ascontiguousarray(Ec[b0:b0 + bpc].transpose(0, 2, 1))
        elif host_et == "hybrid":
            m["EcT"] = np.ascontiguousarray(Ec[b0 + 1:b0 + bpc:2].transpose(0, 2, 1))
        maps.append(m)
    return maps


_cache = {}


def _get_nc(n_c, bpc, host_et, use_f32r, tr_f32r):
    key = (n_c, bpc, host_et, use_f32r, tr_f32r)
    if key not in _cache:
        _cache[key] = _build(n_c, bpc, host_et, use_f32r, tr_f32r)
    return _cache[key]


def run(inputs, trace=False, host_et=HOST_ET, use_f32r=USE_F32R, tr_f32r=TR_F32R):
    from concourse.bass_utils import run_bass_kernel_spmd
    Ec, idx, counts, maskf, n_c = _host_prep(inputs)
    nc = _get_nc(n_c, BPC, host_et, use_f32r, tr_f32r)
    maps = _in_maps(inputs, Ec, maskf, n_c, BPC, host_et, bf16=(use_f32r == "bf16"))
    res = run_bass_kernel_spmd(nc, maps, list(range(NCORES)), trace=trace)
    scores = np.zeros((B, N), dtype=np.float32)
    for i in range(NCORES):
        sc = res.results[i]["scores"]
        for j in range(BPC):
            b = i * BPC + j
            c = counts[b]
            scores[b, idx[b, :c]] = sc[j, :c]
    return scores, res


def kernel(**inputs) -> np.ndarray:
    scores, _ = run(inputs, trace=False)
    return scores



# revision 8
# speedup vs baseline: 1.7521x; 1.7521x over previous
"""Trainium2 Bass kernel for nn_GAT_Decoder (one decode step of a GAT decoder).

Data-parallel over batch: 8 cores x 32 batches. Weights replicated.

Algorithm (O(B*N*D) matmul folding, as v1):
  compat[b,h,n] = qhat[b,h,:] . E[b,n,:]   with qhat = state @ Wq_h @ Wk_h^T
  ctx[b,h,:]    = attn[b,h,:] @ E[b]
  o'[b,(h,d')]  = ctx[b,h,:] @ Wv[:,d']    (extract block-diagonal)
  phat          = o @ G^T,  G = Wk_ptr @ Wo^T (folded on device)
  compat2[b,n]  = phat[b,:] . E[b,n,:]

v2 layout/scheduling:
  - subgroup packing: 4 batches per subgroup, all skinny matmuls issued as
    column-tiled (tile_position=(0,32j)) so 4 batch-chains run CONCURRENTLY
    in the 128x128 PE array (measured 2.9x on HW).
  - E and E^T both streamed from DRAM in bf16, host-swizzled so each SBUF
    partition reads one contiguous run (1 DMA descriptor/partition).
  - softmax/tanh stages operate on [128, n_c] packed tiles (4 batches at
    partition offsets {0,32,64,96}), masks pre-merged into one tile per sg.
  - masked/pad columns get -1e30 logits -> exact 0 scores, matching the
    reference's softmax(-inf)=0; compaction (gather of unmasked cols) is
    done host-side.
"""

import numpy as np

B, N, D, H = 256, 1000, 512, 8
HD = D // H
NCORES = 8
BPC = B // NCORES          # batches per core
SG = 4                     # batches per subgroup
NSG = BPC // SG            # subgroups per core
ND = D // 128              # contraction chunks over D

NORM_MHA = float(1.0 / np.sqrt(HD))
NORM_PTR = float(1.0 / np.sqrt(D))
MASKVAL = 1e30


def _splits(n_c):
    out = [(0, min(512, n_c))]
    if n_c > 512:
        out.append((512, n_c - 512))
    return out


def _build(n_c, bpc):
    from concourse import bacc
    import concourse.mybir as mybir
    import concourse.tile as tile
    from concourse.masks import make_identity

    dt = mybir.dt
    AF = mybir.ActivationFunctionType
    ALU = mybir.AluOpType
    f32 = dt.float32
    bf = dt.bfloat16
    nt = n_c // 128
    nsg = bpc // SG
    spl = _splits(n_c)

    nc = bacc.Bacc("TRN2", target_bir_lowering=False, debug=False)

    # ---- DRAM I/O ----
    Ecs_d = nc.dram_tensor("Ecs", [bpc, 128, nt, D], bf, kind="ExternalInput")
    ETs_d = nc.dram_tensor("ETs", [bpc, 128, ND, n_c], bf, kind="ExternalInput")
    mk_d = nc.dram_tensor("mask32", [nsg, 128, n_c], f32, kind="ExternalInput")
    wfc_d = nc.dram_tensor("W_fc", [D, D], bf, kind="ExternalInput")
    wfc1_d = nc.dram_tensor("W_fc1", [D, D], bf, kind="ExternalInput")
    wq_d = nc.dram_tensor("Wq", [D, D], bf, kind="ExternalInput")
    wkT_d = nc.dram_tensor("WkT", [D, D], bf, kind="ExternalInput")
    wv_d = nc.dram_tensor("Wv", [D, D], bf, kind="ExternalInput")
    woT_d = nc.dram_tensor("WoT", [D, D], bf, kind="ExternalInput")
    wpT_d = nc.dram_tensor("WpT", [D, D], bf, kind="ExternalInput")
    wlastT_d = nc.dram_tensor("wlastT", [128, ND], f32, kind="ExternalInput")
    e0T_d = nc.dram_tensor("E0T", [D, bpc], bf, kind="ExternalInput")
    poolT_d = nc.dram_tensor("poolT", [D, bpc], bf, kind="ExternalInput")
    dcrep_d = nc.dram_tensor("dcrep", [128, bpc], f32, kind="ExternalInput")
    out_d = nc.dram_tensor("scores", [bpc, n_c], f32, kind="ExternalOutput")

    def w_ap(d):  # [512,512] dram -> [128, 4, 512]
        return d[0:D, :].rearrange("(c p) d -> p c d", p=128)

    with tile.TileContext(nc) as tc:
        with tc.tile_pool(name="const", bufs=1) as constp, \
             tc.tile_pool(name="wmain", bufs=1) as wmain:
            ident = constp.tile([128, 128], f32, tag="ident")
            make_identity(nc, ident[:])
            identr = constp.tile([128, 128], bf, tag="identr")
            nc.vector.tensor_copy(identr[:], ident[:])

            # persistent weights
            wv_t = wmain.tile([128, ND, D], bf, tag="wv")
            gt_t = wmain.tile([128, ND, D], bf, tag="gt")
            qhatT = wmain.tile([128, ND, bpc, H], bf, tag="qhatT")
            nc.gpsimd.dma_start(wv_t[:], w_ap(wv_d))

            # ---------- phase 0: weight folds + per-batch query prep ----------
            with tc.tile_pool(name="w0", bufs=1) as w0, \
                 tc.tile_pool(name="ps0", bufs=2, space="PSUM") as ps0:
                wfc_t = w0.tile([128, ND, D], bf, tag="wfc")
                wfc1_t = w0.tile([128, ND, D], bf, tag="wfc1")
                wq_t = w0.tile([128, ND, D], bf, tag="wq")
                wkT_t = w0.tile([128, ND, D], bf, tag="wkT")
                woT_t = w0.tile([128, ND, D], bf, tag="woT")
                wpT_t = w0.tile([128, ND, D], bf, tag="wpT")
                wlast_t = w0.tile([128, ND], f32, tag="wlast")
                e0T_t = w0.tile([128, ND, bpc], bf, tag="e0T")
                poolT_t = w0.tile([128, ND, bpc], bf, tag="poolT")
                dcrep_t = w0.tile([128, bpc], f32, tag="dcrep")
                stateT = w0.tile([128, ND, bpc], bf, tag="stateT")
                qT_t = w0.tile([128, ND, bpc], bf, tag="qT")
                nc.gpsimd.dma_start(wfc_t[:], w_ap(wfc_d))
                nc.gpsimd.dma_start(wfc1_t[:], w_ap(wfc1_d))
                nc.gpsimd.dma_start(wq_t[:], w_ap(wq_d))
                nc.gpsimd.dma_start(wkT_t[:], w_ap(wkT_d))
                nc.gpsimd.dma_start(woT_t[:], w_ap(woT_d))
                nc.gpsimd.dma_start(wpT_t[:], w_ap(wpT_d))
                nc.gpsimd.dma_start(wlast_t[:], wlastT_d[:])
                nc.gpsimd.dma_start(e0T_t[:], e0T_d[:].rearrange("(c p) b -> p c b", p=128))
                nc.gpsimd.dma_start(poolT_t[:], poolT_d[:].rearrange("(c p) b -> p c b", p=128))
                nc.gpsimd.dma_start(dcrep_t[:], dcrep_d[:])

                # G^T = WoT.T @ WpT, scaled by norm_ptr
                for c in range(ND):
                    g_ps = ps0.tile([128, D], f32, tag="g_ps")
                    for kc in range(ND):
                        nc.tensor.matmul(g_ps[:], woT_t[:, kc, 128 * c:128 * (c + 1)],
                                         wpT_t[:, kc, :], start=(kc == 0), stop=(kc == ND - 1))
                    nc.scalar.mul(gt_t[:, c, :], g_ps[:], NORM_PTR)

                # stateT = (W_fc.T @ e0) + (W_fc1.T @ pool) + dc * wlast
                for c in range(ND):
                    st_ps = ps0.tile([128, bpc], f32, tag="st_ps")
                    for kc in range(ND):
                        nc.tensor.matmul(st_ps[:], wfc_t[:, kc, 128 * c:128 * (c + 1)],
                                         e0T_t[:, kc, :], start=(kc == 0), stop=False)
                    for kc in range(ND):
                        nc.tensor.matmul(st_ps[:], wfc1_t[:, kc, 128 * c:128 * (c + 1)],
                                         poolT_t[:, kc, :], start=False, stop=(kc == ND - 1))
                    nc.vector.scalar_tensor_tensor(
                        stateT[:, c, :], dcrep_t[:], wlast_t[:, c:c + 1], st_ps[:],
                        op0=ALU.mult, op1=ALU.add)

                # QT (scaled by norm_mha)
                for c in range(ND):
                    q_ps = ps0.tile([128, bpc], f32, tag="q_ps")
                    for kc in range(ND):
                        nc.tensor.matmul(q_ps[:], wq_t[:, kc, 128 * c:128 * (c + 1)],
                                         stateT[:, kc, :], start=(kc == 0), stop=(kc == ND - 1))
                    nc.scalar.mul(qT_t[:, c, :], q_ps[:], NORM_MHA)

                # qhatT[d, b, h] = sum_{d' in head h} Wk[d, d'] q[b, d']
                for h in range(H):
                    pb = 64 * (h % 2)
                    for c in range(ND):
                        qq = ps0.tile([128, bpc], f32, tag="qq")
                        nc.tensor.matmul(
                            qq[:], wkT_t[pb:pb + 64, h // 2, 128 * c:128 * (c + 1)],
                            qT_t[pb:pb + 64, h // 2, :], start=True, stop=True)
                        nc.vector.tensor_copy(qhatT[:, c, :, h], qq[:])

            # ---------- main loop: one subgroup (4 batches) per round ----------
            with tc.tile_pool(name="epool", bufs=12) as epool, \
                 tc.tile_pool(name="etpool", bufs=16) as etpool, \
                 tc.tile_pool(name="mkpool", bufs=4) as mkpool, \
                 tc.tile_pool(name="sgp", bufs=2) as sgp, \
                 tc.tile_pool(name="smallp", bufs=4) as smallp, \
                 tc.tile_pool(name="ps_a", bufs=4, space="PSUM") as ps_a, \
                 tc.tile_pool(name="ps_tp", bufs=2, space="PSUM") as ps_tp, \
                 tc.tile_pool(name="ps_b", bufs=2, space="PSUM") as ps_b:
                st = {}

                def emit_dma(s):
                    es, ets = [], []
                    for j in range(SG):
                        e_t = epool.tile([128, nt, D], bf, tag="E", name=f"E{s}_{j}")
                        nc.sync.dma_start(e_t[:], Ecs_d[SG * s + j])
                        es.append(e_t)
                        et_t = etpool.tile([128, ND, n_c], bf, tag="ET", name=f"ET{s}_{j}")
                        nc.gpsimd.dma_start(et_t[:], ETs_d[SG * s + j])
                        ets.append(et_t)
                    mk = mkpool.tile([128, n_c], f32, tag="mk", name=f"mk{s}")
                    nc.scalar.dma_start(mk[:], mk_d[s])
                    st[s] = dict(e=es, et=ets, mk=mk)

                def emit_A2(s):
                    cps = []
                    for si, (f0, fl) in enumerate(spl):
                        cp = ps_a.tile([128, fl], f32, tag="cp", name=f"cp{s}_{si}")
                        cps.append(cp)
                    for c in range(ND):
                        for j in range(SG):
                            et = st[s]['et'][j]
                            for (f0, fl), cp in zip(spl, cps):
                                nc.tensor.matmul(
                                    cp[32 * j:32 * j + 8, 0:fl],
                                    qhatT[:, c, SG * s + j, :],
                                    et[:, c, f0:f0 + fl],
                                    start=(c == 0), stop=(c == ND - 1),
                                    tile_position=(0, 32 * j))
                    st[s]['cps'] = cps

                def emit_soft(s):
                    cps, mk = st[s]['cps'], st[s]['mk']
                    u8 = sgp.tile([128, n_c], f32, tag="u8", name=f"u8{s}")
                    for (f0, fl), cp in zip(spl, cps):
                        nc.vector.tensor_sub(u8[:, f0:f0 + fl], cp[:, 0:fl],
                                             mk[:, f0:f0 + fl])
                    s8 = smallp.tile([128, 1], f32, tag="s8", name=f"s8{s}")
                    ex = sgp.tile([128, n_c], bf, tag="ex", name=f"ex{s}")
                    nc.scalar.activation(ex[:, :], u8[:, :], AF.Exp,
                                         bias=0.0, scale=1.0, accum_out=s8[:])
                    r8 = smallp.tile([128, 1], f32, tag="r8", name=f"r8{s}")
                    nc.vector.reciprocal(r8[:], s8[:])
                    st[s].update(ex=ex, r8=r8)

                def emit_expT(s):
                    ex = st[s]['ex']
                    expT = sgp.tile([128, nt, 128], bf, tag="expT", name=f"expT{s}")
                    for t in range(nt):
                        tp = ps_tp.tile([128, 128], bf, tag="tp", name=f"tpE{s}_{t}")
                        nc.tensor.transpose(tp[:], ex[:, 128 * t:128 * (t + 1)], identr[:])
                        if t % 2 == 0:
                            nc.vector.tensor_copy(expT[:, t, :], tp[:])
                        else:
                            nc.scalar.copy(expT[:, t, :], tp[:])
                    st[s]['expT'] = expT

                def emit_ctxu(s):
                    expT, r8 = st[s]['expT'], st[s]['r8']
                    cxp = ps_b.tile([128, D], f32, tag="bigb", name=f"cxp{s}")
                    for t in range(nt):
                        for j in range(SG):
                            nc.tensor.matmul(
                                cxp[32 * j:32 * j + 8, :],
                                expT[:, t, 32 * j:32 * j + 8],
                                st[s]['e'][j][:, t, :],
                                start=(t == 0), stop=(t == nt - 1),
                                tile_position=(0, 32 * j))
                    ctx8 = sgp.tile([128, D], bf, tag="ctx8", name=f"ctx8{s}")
                    nc.scalar.activation(ctx8[:, :], cxp[:, :], AF.Copy,
                                         bias=0.0, scale=r8[:])
                    st[s]['ctx8'] = ctx8

                def emit_oC1(s):
                    ctx8 = st[s]['ctx8']
                    # ctxT
                    ctxT = sgp.tile([128, ND, 128], bf, tag="ctxT", name=f"ctxT{s}")
                    for c in range(ND):
                        tp = ps_tp.tile([128, 128], bf, tag="tp", name=f"tpC{s}_{c}")
                        nc.tensor.transpose(tp[:], ctx8[:, 128 * c:128 * (c + 1)], identr[:])
                        if c % 2 == 0:
                            nc.vector.tensor_copy(ctxT[:, c, :], tp[:])
                        else:
                            nc.scalar.copy(ctxT[:, c, :], tp[:])
                    # o' packed: all (h, d') products; block-diagonal extracted below
                    o_ps = ps_b.tile([128, D], f32, tag="bigb", name=f"o{s}")
                    for c in range(ND):
                        for j in range(SG):
                            nc.tensor.matmul(
                                o_ps[32 * j:32 * j + 8, :],
                                ctxT[:, c, 32 * j:32 * j + 8],
                                wv_t[:, c, :],
                                start=(c == 0), stop=(c == ND - 1),
                                tile_position=(0, 32 * j))
                    o8 = sgp.tile([128, D], bf, tag="o8", name=f"o8{s}")
                    nc.scalar.copy(o8[:, :], o_ps[:, :])
                    # oT + block-diagonal extract: oT[d' in c2, j] needs head
                    # h=2*c2 (rows 0:64) / h=2*c2+1 (rows 64:128)
                    oT = sgp.tile([128, ND, SG], bf, tag="oT", name=f"oT{s}")
                    for c2 in range(ND):
                        tp = ps_tp.tile([128, 128], bf, tag="tp", name=f"tpO{s}_{c2}")
                        nc.tensor.transpose(tp[:], o8[:, 128 * c2:128 * (c2 + 1)], identr[:])
                        tpr = tp[:, :].rearrange("p (j i) -> p i j", i=32)
                        nc.vector.tensor_copy(oT[0:64, c2, :], tpr[0:64, 2 * c2, :])
                        nc.vector.tensor_copy(oT[64:128, c2, :], tpr[64:128, 2 * c2 + 1, :])
                    # phat^T = gt.T-chunks @ oT   -> [128(d'-chunk), (c2, j)]
                    pp = ps_b.tile([128, SG * ND], f32, tag="bigb", name=f"pp{s}")
                    for c2 in range(ND):
                        for c in range(ND):
                            nc.tensor.matmul(
                                pp[:, SG * c2:SG * (c2 + 1)],
                                gt_t[:, c, 128 * c2:128 * (c2 + 1)],
                                oT[:, c, :], start=(c == 0), stop=(c == ND - 1))
                    phatT = sgp.tile([128, ND, SG], bf, tag="phatT", name=f"phatT{s}")
                    nc.vector.tensor_copy(phatT[:, :, :],
                                          pp[:, :].rearrange("p (c j) -> p c j", c=ND))
                    st[s]['phatT'] = phatT

                def emit_C2(s):
                    phatT = st[s]['phatT']
                    qs = []
                    for si, (f0, fl) in enumerate(spl):
                        q = ps_a.tile([128, fl], f32, tag="cp", name=f"q{s}_{si}")
                        qs.append(q)
                    for c in range(ND):
                        for j in range(SG):
                            et = st[s]['et'][j]
                            for (f0, fl), q in zip(spl, qs):
                                nc.tensor.matmul(
                                    q[32 * j:32 * j + 1, 0:fl],
                                    phatT[:, c, j:j + 1],
                                    et[:, c, f0:f0 + fl],
                                    start=(c == 0), stop=(c == ND - 1),
                                    tile_position=(0, 32 * j))
                    st[s]['qs'] = qs

                def emit_ptr(s):
                    qs, mk = st[s]['qs'], st[s]['mk']
                    tn = sgp.tile([128, n_c], f32, tag="tn", name=f"tn{s}")
                    for (f0, fl), q in zip(spl, qs):
                        nc.scalar.activation(tn[:, f0:f0 + fl], q[:, 0:fl], AF.Tanh)
                    x = sgp.tile([128, n_c], f32, tag="x", name=f"x{s}")
                    nc.vector.scalar_tensor_tensor(
                        x[:, :], tn[:, :], 10.0, mk[:, :],
                        op0=mybir.AluOpType.mult, op1=mybir.AluOpType.subtract)
                    s2 = smallp.tile([128, 1], f32, tag="s2", name=f"s2{s}")
                    e2 = sgp.tile([128, n_c], f32, tag="e2", name=f"e2{s}")
                    nc.scalar.activation(e2[:, :], x[:, :], AF.Exp,
                                         bias=0.0, scale=1.0, accum_out=s2[:])
                    r2 = smallp.tile([128, 1], f32, tag="r2", name=f"r2{s}")
                    nc.vector.reciprocal(r2[:], s2[:])
                    sc = sgp.tile([128, n_c], f32, tag="sc", name=f"sc{s}")
                    nc.vector.tensor_scalar_mul(sc[:, :], e2[:, :], r2[:])
                    for j in range(SG):
                        nc.sync.dma_start(out_d[SG * s + j], sc[32 * j:32 * j + 1, :])
                    del st[s]

                LAG = 2
                for r in range(nsg + LAG):
                    if r < nsg:
                        emit_dma(r)
                    s = r - LAG
                    if 0 <= s < nsg:
                        emit_A2(s)
                        emit_soft(s)
                        if s - 1 >= 0:
                            emit_oC1(s - 1)
                        emit_expT(s)
                        emit_ctxu(s)
                        if s - 1 >= 0:
                            emit_C2(s - 1)
                            emit_ptr(s - 1)
                        if s == nsg - 1:
                            emit_oC1(s)
                            emit_C2(s)
                            emit_ptr(s)

    nc.finalize()
    return nc


def _host_prep(inputs):
    E = np.ascontiguousarray(inputs['encoder_inputs'], dtype=np.float32)
    mask = np.asarray(inputs['mask'])
    unm = (mask == 0)
    counts = unm.sum(axis=1)
    n_c = max(512, int(np.ceil(counts.max() / 128) * 128))
    idx = np.zeros((B, n_c), dtype=np.int64)
    maskf = np.full((B, n_c), MASKVAL, dtype=np.float32)
    for b in range(B):
        ii = np.nonzero(unm[b])[0]
        k = min(len(ii), n_c)
        idx[b, :k] = ii[:k]
        maskf[b, :k] = 0.0
    Ec = np.take_along_axis(E, idx[:, :, None], axis=1)   # [B, n_c, D] f32
    return Ec, idx, counts, maskf, n_c


def _in_maps(inputs, Ec, maskf, n_c):
    import ml_dtypes
    bf16 = ml_dtypes.bfloat16
    nt = n_c // 128
    W_fc = np.asarray(inputs['W_fc'], dtype=np.float32)
    wlastT = np.ascontiguousarray(W_fc[D].reshape(ND, 128).T)        # [128, ND]
    com = {
        "W_fc": W_fc[:D].astype(bf16),
        "W_fc1": np.asarray(inputs['W_fc1'], dtype=np.float32).astype(bf16),
        "Wq": np.asarray(inputs['Wq'], dtype=np.float32).astype(bf16),
        "WkT": np.ascontiguousarray(np.asarray(inputs['Wk_mha']).T).astype(bf16),
        "Wv": np.asarray(inputs['Wv'], dtype=np.float32).astype(bf16),
        "WoT": np.ascontiguousarray(np.asarray(inputs['Wo']).T).astype(bf16),
        "WpT": np.ascontiguousarray(np.asarray(inputs['Wk_ptr']).T).astype(bf16),
        "wlastT": wlastT,
    }
    pool = np.asarray(inputs['pool'], dtype=np.float32)
    dc = np.asarray(inputs['dynamic_capacity'], dtype=np.float32)
    maps = []
    for i in range(NCORES):
        b0 = i * BPC
        Ecc = Ec[b0:b0 + BPC].astype(bf16)                  # [bpc, n_c, D]
        # swizzles: one contiguous run per SBUF partition
        Ecs = np.ascontiguousarray(
            Ecc.reshape(BPC, nt, 128, D).transpose(0, 2, 1, 3))
        ETs = np.ascontiguousarray(
            Ecc.transpose(0, 2, 1).reshape(BPC, ND, 128, n_c).transpose(0, 2, 1, 3))
        # merged mask tile: batch j of each sg replicated at partitions
        # 32j..32j+7 (MHA) and used at row 32j for the pointer stage;
        # all other partitions killed with MASKVAL
        m32 = np.full((NSG, 128, n_c), MASKVAL, dtype=np.float32)
        mf = maskf[b0:b0 + BPC].reshape(NSG, SG, n_c)
        for j in range(SG):
            m32[:, 32 * j:32 * j + 8, :] = mf[:, j:j + 1, :]
        m = dict(com)
        m.update({
            "Ecs": Ecs,
            "ETs": ETs,
            "mask32": m32,
            "E0T": np.ascontiguousarray(Ec[b0:b0 + BPC, 0, :].T).astype(bf16),
            "poolT": np.ascontiguousarray(pool[b0:b0 + BPC].T).astype(bf16),
            "dcrep": np.ascontiguousarray(
                np.broadcast_to(dc[b0:b0 + BPC, 0], (128, BPC))),
        })
        maps.append(m)
    return maps


_cache = {}


def _get_nc(n_c, bpc):
    key = (n_c, bpc)
    if key not in _cache:
        _cache[key] = _build(n_c, bpc)
    return _cache[key]


def run(inputs, trace=False, **_ignored):
    from concourse.bass_utils import run_bass_kernel_spmd
    Ec, idx, counts, maskf, n_c = _host_prep(inputs)
    nc = _get_nc(n_c, BPC)
    maps = _in_maps(inputs, Ec, maskf, n_c)
    res = run_bass_kernel_spmd(nc, maps, list(range(NCORES)), trace=trace)
    scores = np.zeros((B, N), dtype=np.float32)
    for i in range(NCORES):
        sc = res.results[i]["scores"]
        for j in range(BPC):
            b = i * BPC + j
            c = counts[b]
            scores[b, idx[b, :c]] = sc[j, :c]
    return scores, res


def kernel(**inputs) -> np.ndarray:
    scores, _ = run(inputs, trace=False)
    return scores
